# revision 7
# baseline (speedup 1.0000x reference)
"""Clifford predictive-coding network on 8 TRN2 NeuronCores.

Problem: Cl(3,0) geometric-product PC net, layers [256, 512, 256], batch 64,
n_iter PC iterations:
    for l in 1,2:
        x_hat = gp(states[l], w_l)                 # contract over D_l
        err   = states[l-1] - x_hat
        states[l] += alpha * gp(err, transpose(w_l * REV))

Sharding: pure data-parallel over batch (8 cores x batch-8); weights and the
Cayley table are replicated; no cross-core communication.

Per-core kernel design ("arrangement A"):
  Every tensor lives in layout L1 = (feature dim on partitions in 128-chunks,
  64 free columns, col = 8*k + b) for blade k in 0..7, local batch b in 0..7.
  A geometric product out[b,m,k] = sum_{n,a,c} C[a,c,k] A[b,n,a] W[m,n,c]
  becomes, for each blade c and each 128-chunk (nj, mi):
      psum[mi][:, view] += lhsT(W~[c,nj,mi]).T @ rhs_view(state, c, nj)
  where rhs_view is an access-pattern view of the state tile that applies the
  XOR column permutation k -> k^c and the Clifford sign: the state is stored
  as [+S | -S] (128 cols per chunk) and the sign s(k,c) becomes a +64 column
  offset.  sign(k^c,c) = sigma(c) * (-1)^{k . u(c)} is affine in the bits of
  k for 6 of 8 blades; blades needing >3 AP dims (ISA limit) are split into
  two half-matmuls by k2 (contiguous psum halves).

  Master states are kept in fp32; matmul operands are bf16; alpha and the
  reversion signs are folded into the backward weight tiles on the host.
"""

import numpy as np
import ml_dtypes

import concourse.bass as bass
import concourse.mybir as mybir
import concourse.tile as tile
from concourse import bacc
from concourse.tile import add_dep_helper
from concourse import bass_utils

# ---------------------------------------------------------------- problem dims
NB = 8          # blades of Cl(3,0)
D0, D1, D2 = 256, 512, 256
BATCH = 64
NCORES = 8
BLOC = BATCH // NCORES          # 8 local batch
ALPHA = 0.1
F = 64                          # free columns: 8 blades x 8 batch


def _cayley():
    C = np.zeros((NB, NB, NB), dtype=np.float32)
    for a in range(NB):
        for b in range(NB):
            s, aa = 0, a >> 1
            while aa:
                s += bin(aa & b).count("1")
                aa >>= 1
            C[a, b, a ^ b] = -1.0 if (s & 1) else 1.0
    return C


CAYLEY = _cayley()
REV = np.array(
    [(-1.0) ** (bin(k).count("1") * (bin(k).count("1") - 1) // 2) for k in range(NB)],
    np.float32,
)

# ------------------------------------------------------- AP view construction


def _fit_affine(addr, klist):
    a = [addr[k] for k in klist]
    if len(klist) == 8:
        s0, s1, s2 = a[1] - a[0], a[2] - a[0], a[4] - a[0]
        for i in range(8):
            if a[i] != a[0] + ((i >> 2) & 1) * s2 + ((i >> 1) & 1) * s1 + (i & 1) * s0:
                return None
        return [(s2, 2), (s1, 2), (s0, 2)]
    s0, sg = a[1] - a[0], a[2] - a[0]
    for i in range(4):
        if a[i] != a[0] + ((i >> 1) & 1) * sg + (i & 1) * s0:
            return None
    return [(sg, 2), (s0, 2)]


def _merge(dims):
    out = list(dims)
    changed = True
    while changed:
        changed = False
        for i in range(len(out) - 1):
            (so, no), (si, ni) = out[i], out[i + 1]
            if so == si * ni:
                out[i : i + 2] = [(si, no * ni)]
                changed = True
                break
    return out


def gp_view_specs(c):
    """views for blade c: list of (src_off, src_dims, out_off, out_len)."""
    s = [1 if CAYLEY[k ^ c, c, k] < 0 else 0 for k in range(NB)]
    addr = {k: 64 * s[k] + 8 * (k ^ c) for k in range(NB)}
    full = _fit_affine(addr, list(range(8)))
    if full is not None:
        dims = _merge(list(full) + [(1, 8)])
        if len(dims) <= 3:
            return [(addr[0], dims, 0, 64)]
    views = []
    for h in (0, 1):
        klist = [4 * h + j for j in range(4)]
        sf = _fit_affine(addr, klist)
        assert sf is not None
        dims = _merge(list(sf) + [(1, 8)])
        assert len(dims) <= 3, (c, h, dims)
        views.append((addr[klist[0]], dims, 32 * h, 32))
    return views


VIEW_SPECS = [gp_view_specs(c) for c in range(NB)]


def _make_ap(base_ap, extra_off, dims):
    ap_pairs = [list(base_ap.ap[0])] + [[st, n] for (st, n) in dims]
    return bass.AP(
        tensor=base_ap.tensor, offset=base_ap.offset + extra_off, ap=ap_pairs
    )


def emit_gp(nc, ps_ap, st_ap, lhsT_of, njc):
    """psum (128,64) <- sum over (nj, c) of signed-permuted matmuls.

    st_ap: base AP of the +/- state tile (128, njc*128); block nj at col nj*128.
    lhsT_of(c, nj): (128,128) weight slice AP.
    """
    specs = [
        (c, nj, v) for nj in range(njc) for c in range(NB) for v in VIEW_SPECS[c]
    ]
    n = len(specs)
    for i, (c, nj, (soff, sdims, ooff, olen)) in enumerate(specs):
        rhs = _make_ap(st_ap, nj * 128 + soff, sdims)
        nc.tensor.matmul(
            ps_ap[:, ooff : ooff + olen],
            lhsT_of(c, nj),
            rhs,
            start=(i == 0),
            stop=(i == n - 1),
        )


# ------------------------------------------------------------- bass program


def build_program(n_iter, wdt=mybir.dt.bfloat16):
    f32 = mybir.dt.float32
    nc = bacc.Bacc("TRN2", target_bir_lowering=False, debug=False)

    # dram I/O (per-core shapes)
    xt_d = nc.dram_tensor("xt", (2, 128, F), f32, kind="ExternalInput")
    w1f_d = nc.dram_tensor("w1f", (NB, 4, 2, 128, 128), wdt, kind="ExternalInput")
    w1b_d = nc.dram_tensor("w1b", (NB, 2, 4, 128, 128), wdt, kind="ExternalInput")
    w2f_d = nc.dram_tensor("w2f", (NB, 2, 4, 128, 128), wdt, kind="ExternalInput")
    w2b_d = nc.dram_tensor("w2b", (NB, 4, 2, 128, 128), wdt, kind="ExternalInput")
    s1o_d = nc.dram_tensor("s1o", (4, 128, F), f32, kind="ExternalOutput")
    s2o_d = nc.dram_tensor("s2o", (2, 128, F), f32, kind="ExternalOutput")

    with tile.TileContext(nc) as tc:
        # resident sbuf tensors
        w1f = nc.alloc_sbuf_tensor("w1f_s", [128, NB * 4 * 2 * 128], wdt)
        w1b = nc.alloc_sbuf_tensor("w1b_s", [128, NB * 2 * 4 * 128], wdt)
        w2f = nc.alloc_sbuf_tensor("w2f_s", [128, NB * 2 * 4 * 128], wdt)
        w2b = nc.alloc_sbuf_tensor("w2b_s", [128, NB * 4 * 2 * 128], wdt)
        xt = nc.alloc_sbuf_tensor("xt_s", [128, 2 * F], f32)
        s1m = nc.alloc_sbuf_tensor("s1m", [128, 4 * F], f32)
        s2m = nc.alloc_sbuf_tensor("s2m", [128, 2 * F], f32)
        s1pm = nc.alloc_sbuf_tensor("s1pm", [128, 4 * 128], wdt)
        s2pm = nc.alloc_sbuf_tensor("s2pm", [128, 2 * 128], wdt)
        e1pm = nc.alloc_sbuf_tensor("e1pm", [128, 2 * 128], wdt)
        e2pm = nc.alloc_sbuf_tensor("e2pm", [128, 4 * 128], wdt)

        def wslice(t, c, nj, mi, njc, mic):
            off = ((c * njc + nj) * mic + mi) * 128
            return t[:, off : off + 128]

        # x first, as a single DMA (one queue-sem tick for the DVE probe below)
        nc.sync.dma_start(
            out=xt[:].rearrange("p (n f) -> p n f", f=F),
            in_=xt_d.ap().transpose([1, 0, 2]),
        )
        # DVE probe: absorbs the DMA wait into the vector engine's clock so
        # later DVE ops touching xt need only one (PE) wait -- the ISA
        # TensorTensor slot fits a single sync-wait command.
        probe = nc.alloc_sbuf_tensor("probe", [128, 1], mybir.dt.float32)
        p_op = nc.vector.tensor_copy(out=probe[:], in_=xt[:, 0:1])

        # weight DMAs (also warms SBUF while first matmuls only need c=0 slices)
        for d, s, njc, mic in (
            (w1f_d, w1f, 4, 2),
            (w1b_d, w1b, 2, 4),
            (w2f_d, w2f, 2, 4),
            (w2b_d, w2b, 4, 2),
        ):
            for c in range(NB):
                for nj in range(njc):
                    for mi in range(mic):
                        nc.sync.dma_start(
                            out=wslice(s, c, nj, mi, njc, mic), in_=d[c, nj, mi]
                        )

        for t in (s1m, s2m, s1pm, s2pm):
            ms = nc.vector.memset(t[:], 0.0)
            add_dep_helper(ms.ins, p_op.ins, sync=False, reason="probe first")

        def blk(t, mi):  # 64-col fp32 block
            return t[:, mi * F : (mi + 1) * F]

        def pmp(t, mi):  # + half of a +/- block
            return t[:, mi * 128 : mi * 128 + 64]

        def pmm(t, mi):  # - half
            return t[:, mi * 128 + 64 : mi * 128 + 128]

        with tc.tile_pool(name="ps", bufs=2, space="PSUM") as pp:
            for _ in range(n_iter):
                # ---- layer 1 forward: xhat1 = gp(S1, w1f); err1 = x - xhat1
                for mi in range(2):
                    ps = pp.tile([128, F], f32, tag="xh1")
                    emit_gp(
                        nc, ps[:], s1pm[:], lambda c, nj: wslice(w1f, c, nj, mi, 4, 2), 4
                    )
                    nc.vector.tensor_sub(pmp(e1pm, mi), blk(xt, mi), ps[:])
                    nc.vector.tensor_sub(pmm(e1pm, mi), ps[:], blk(xt, mi))
                # ---- layer 1 backward: S1 += alpha * gp(err1, w1bT)
                for mi in range(4):
                    ps = pp.tile([128, F], f32, tag="ds1")
                    emit_gp(
                        nc, ps[:], e1pm[:], lambda c, nj: wslice(w1b, c, nj, mi, 2, 4), 2
                    )
                    nc.vector.tensor_add(blk(s1m, mi), blk(s1m, mi), ps[:])
                    nc.vector.tensor_copy(out=pmp(s1pm, mi), in_=blk(s1m, mi))
                    nc.vector.tensor_scalar_mul(pmm(s1pm, mi), blk(s1m, mi), -1.0)
                # ---- layer 2 forward: xhat2 = gp(S2, w2f); err2 = S1 - xhat2
                for mi in range(4):
                    ps = pp.tile([128, F], f32, tag="xh2")
                    emit_gp(
                        nc, ps[:], s2pm[:], lambda c, nj: wslice(w2f, c, nj, mi, 2, 4), 2
                    )
                    nc.vector.tensor_sub(pmp(e2pm, mi), blk(s1m, mi), ps[:])
                    nc.vector.tensor_sub(pmm(e2pm, mi), ps[:], blk(s1m, mi))
                # ---- layer 2 backward: S2 += alpha * gp(err2, w2bT)
                for mi in range(2):
                    ps = pp.tile([128, F], f32, tag="ds2")
                    emit_gp(
                        nc, ps[:], e2pm[:], lambda c, nj: wslice(w2b, c, nj, mi, 4, 2), 4
                    )
                    nc.vector.tensor_add(blk(s2m, mi), blk(s2m, mi), ps[:])
                    nc.vector.tensor_copy(out=pmp(s2pm, mi), in_=blk(s2m, mi))
                    nc.vector.tensor_scalar_mul(pmm(s2pm, mi), blk(s2m, mi), -1.0)

        for nj in range(4):
            nc.sync.dma_start(out=s1o_d[nj], in_=blk(s1m, nj))
        for nj in range(2):
            nc.sync.dma_start(out=s2o_d[nj], in_=blk(s2m, nj))

    nc.compile()
    return nc


# ------------------------------------------------------------ host wrappers

_CACHE = {}


def _get_program(n_iter, wdt):
    key = (n_iter, wdt)
    if key not in _CACHE:
        _CACHE[key] = build_program(n_iter, wdt)
    return _CACHE[key]


def _prep_weights(w1, w2, npdt):
    a = np.float32(ALPHA)
    # W1f[c,nj,mi,p,q] = w1[mi*128+q, nj*128+p, c]
    w1f = np.ascontiguousarray(
        w1.transpose(2, 1, 0).reshape(NB, 4, 128, 2, 128).transpose(0, 1, 3, 2, 4)
    )
    # W1b[c,nj,mi,p,q] = a*REV[c]*w1[nj*128+p, mi*128+q, c]
    w1b = (a * REV)[:, None, None, None, None] * w1.transpose(2, 0, 1).reshape(
        NB, 2, 128, 4, 128
    ).transpose(0, 1, 3, 2, 4)
    # W2f[c,nj,mi,p,q] = w2[mi*128+q, nj*128+p, c]
    w2f = np.ascontiguousarray(
        w2.transpose(2, 1, 0).reshape(NB, 2, 128, 4, 128).transpose(0, 1, 3, 2, 4)
    )
    # W2b[c,nj,mi,p,q] = a*REV[c]*w2[nj*128+p, mi*128+q, c]
    w2b = (a * REV)[:, None, None, None, None] * w2.transpose(2, 0, 1).reshape(
        NB, 4, 128, 2, 128
    ).transpose(0, 1, 3, 2, 4)
    return [np.ascontiguousarray(t).astype(npdt) for t in (w1f, w1b, w2f, w2b)]


def _run(x, w1, w2, n_iter, trace=False, use_bf16=True):
    x = np.asarray(x, np.float32)
    w1 = np.asarray(w1, np.float32)
    w2 = np.asarray(w2, np.float32)
    n_iter = int(np.asarray(n_iter))

    wdt = mybir.dt.bfloat16 if use_bf16 else mybir.dt.float32
    npdt = ml_dtypes.bfloat16 if use_bf16 else np.float32
    nc = _get_program(n_iter, wdt)

    w1f, w1b, w2f, w2b = _prep_weights(w1, w2, npdt)

    in_maps = []
    for core in range(NCORES):
        xc = x[core * BLOC : (core + 1) * BLOC]          # (8, 256, 8)
        xtc = np.ascontiguousarray(
            xc.transpose(1, 2, 0).reshape(2, 128, F)
        )  # [nj,p,8k+b]
        in_maps.append(
            {"xt": xtc, "w1f": w1f, "w1b": w1b, "w2f": w2f, "w2b": w2b}
        )

    res = bass_utils.run_bass_kernel_spmd(
        nc, in_maps, core_ids=list(range(NCORES)), trace=trace
    )

    s1 = np.zeros((BATCH, D1, NB), np.float32)
    s2 = np.zeros((BATCH, D2, NB), np.float32)
    for core in range(NCORES):
        r = res.results[core]
        # s1o (4,128,64): [nj,p,8k+b] -> (b, nj*128+p, k)
        s1[core * BLOC : (core + 1) * BLOC] = (
            r["s1o"].reshape(4, 128, NB, BLOC).transpose(3, 0, 1, 2).reshape(BLOC, D1, NB)
        )
        s2[core * BLOC : (core + 1) * BLOC] = (
            r["s2o"].reshape(2, 128, NB, BLOC).transpose(3, 0, 1, 2).reshape(BLOC, D2, NB)
        )
    return (x, s1, s2), res


def kernel(x, w1, w2, n_iter):
    (x, s1, s2), _ = _run(x, w1, w2, n_iter)
    return (x, s1, s2)


# revision 12
# speedup vs baseline: 1.4089x; 1.4089x over previous
"""Clifford predictive-coding network on 8 TRN2 NeuronCores.

Problem: Cl(3,0) geometric-product PC net, layers [256, 512, 256], batch 64,
n_iter PC iterations:
    for l in 1,2:
        x_hat = gp(states[l], w_l)                 # contract over D_l
        err   = states[l-1] - x_hat
        states[l] += alpha * gp(err, transpose(w_l * REV))

Sharding: pure data-parallel over batch (8 cores x batch-8); weights and the
Cayley table are replicated; no cross-core communication.

Per-core kernel design ("arrangement A"):
  Every tensor lives in layout L1 = (feature dim on partitions in 128-chunks,
  64 free columns, col = 8*k + b) for blade k in 0..7, local batch b in 0..7.
  A geometric product out[b,m,k] = sum_{n,a,c} C[a,c,k] A[b,n,a] W[m,n,c]
  becomes, for each blade c and each 128-chunk (nj, mi):
      psum[mi][:, view] += lhsT(W~[c,nj,mi]).T @ rhs_view(state, c, nj)
  where rhs_view is an access-pattern view of the state tile that applies the
  XOR column permutation k -> k^c and the Clifford sign: the state is stored
  as [+S | -S] (128 cols per chunk) and the sign s(k,c) becomes a +64 column
  offset.  sign(k^c,c) = sigma(c) * (-1)^{k . u(c)} is affine in the bits of
  k for 6 of 8 blades; blades needing >3 AP dims (ISA limit) are split into
  two half-matmuls by k2 (contiguous psum halves).

  Master states are kept in fp32; matmul operands are bf16; alpha and the
  reversion signs are folded into the backward weight tiles on the host.
"""

import numpy as np
import ml_dtypes

import concourse.bass as bass
import concourse.mybir as mybir
import concourse.tile as tile
from concourse import bacc
from concourse.tile import add_dep_helper
from concourse import bass_utils

# ---------------------------------------------------------------- problem dims
NB = 8          # blades of Cl(3,0)
D0, D1, D2 = 256, 512, 256
BATCH = 64
NCORES = 8
BLOC = BATCH // NCORES          # 8 local batch
ALPHA = 0.1
F = 64                          # free columns: 8 blades x 8 batch


def _cayley():
    C = np.zeros((NB, NB, NB), dtype=np.float32)
    for a in range(NB):
        for b in range(NB):
            s, aa = 0, a >> 1
            while aa:
                s += bin(aa & b).count("1")
                aa >>= 1
            C[a, b, a ^ b] = -1.0 if (s & 1) else 1.0
    return C


CAYLEY = _cayley()
REV = np.array(
    [(-1.0) ** (bin(k).count("1") * (bin(k).count("1") - 1) // 2) for k in range(NB)],
    np.float32,
)

# ------------------------------------------------------- AP view construction


def _fit_affine(addr, klist):
    a = [addr[k] for k in klist]
    if len(klist) == 8:
        s0, s1, s2 = a[1] - a[0], a[2] - a[0], a[4] - a[0]
        for i in range(8):
            if a[i] != a[0] + ((i >> 2) & 1) * s2 + ((i >> 1) & 1) * s1 + (i & 1) * s0:
                return None
        return [(s2, 2), (s1, 2), (s0, 2)]
    s0, sg = a[1] - a[0], a[2] - a[0]
    for i in range(4):
        if a[i] != a[0] + ((i >> 1) & 1) * sg + (i & 1) * s0:
            return None
    return [(sg, 2), (s0, 2)]


def _merge(dims):
    out = list(dims)
    changed = True
    while changed:
        changed = False
        for i in range(len(out) - 1):
            (so, no), (si, ni) = out[i], out[i + 1]
            if so == si * ni:
                out[i : i + 2] = [(si, no * ni)]
                changed = True
                break
    return out


def gp_view_specs(c):
    """views for blade c: list of (src_off, src_dims, out_off, out_len)."""
    s = [1 if CAYLEY[k ^ c, c, k] < 0 else 0 for k in range(NB)]
    addr = {k: 64 * s[k] + 8 * (k ^ c) for k in range(NB)}
    full = _fit_affine(addr, list(range(8)))
    if full is not None:
        dims = _merge(list(full) + [(1, 8)])
        if len(dims) <= 3:
            return [(addr[0], dims, 0, 64)]
    views = []
    for h in (0, 1):
        klist = [4 * h + j for j in range(4)]
        sf = _fit_affine(addr, klist)
        assert sf is not None
        dims = _merge(list(sf) + [(1, 8)])
        assert len(dims) <= 3, (c, h, dims)
        views.append((addr[klist[0]], dims, 32 * h, 32))
    return views


VIEW_SPECS = [gp_view_specs(c) for c in range(NB)]


def _make_ap(base_ap, extra_off, dims):
    ap_pairs = [list(base_ap.ap[0])] + [[st, n] for (st, n) in dims]
    return bass.AP(
        tensor=base_ap.tensor, offset=base_ap.offset + extra_off, ap=ap_pairs
    )


def emit_gp(nc, ps_ap, st_ap, lhsT_of, njc):
    """psum (128,64) <- sum over (nj, c) of signed-permuted matmuls.

    st_ap: base AP of the +/- state tile (128, njc*128); block nj at col nj*128.
    lhsT_of(c, nj): (128,128) weight slice AP.
    """
    specs = [
        (c, nj, v) for nj in range(njc) for c in range(NB) for v in VIEW_SPECS[c]
    ]
    n = len(specs)
    for i, (c, nj, (soff, sdims, ooff, olen)) in enumerate(specs):
        rhs = _make_ap(st_ap, nj * 128 + soff, sdims)
        nc.tensor.matmul(
            ps_ap[:, ooff : ooff + olen],
            lhsT_of(c, nj),
            rhs,
            start=(i == 0),
            stop=(i == n - 1),
        )


# ------------------------------------------------------------- bass program


def build_program(n_iter, wdt=mybir.dt.bfloat16):
    f32 = mybir.dt.float32
    nc = bacc.Bacc("TRN2", target_bir_lowering=False, debug=False)

    # dram I/O (per-core shapes)
    xt_d = nc.dram_tensor("xt", (2, 128, F), f32, kind="ExternalInput")
    w1f_d = nc.dram_tensor("w1f", (NB, 4, 2, 128, 128), wdt, kind="ExternalInput")
    w1b_d = nc.dram_tensor("w1b", (NB, 2, 4, 128, 128), wdt, kind="ExternalInput")
    w2f_d = nc.dram_tensor("w2f", (NB, 2, 4, 128, 128), wdt, kind="ExternalInput")
    w2b_d = nc.dram_tensor("w2b", (NB, 4, 2, 128, 128), wdt, kind="ExternalInput")
    s1o_d = nc.dram_tensor("s1o", (4, 128, F), f32, kind="ExternalOutput")
    s2o_d = nc.dram_tensor("s2o", (2, 128, F), f32, kind="ExternalOutput")

    with tile.TileContext(nc) as tc:
        # resident sbuf tensors
        w1f = nc.alloc_sbuf_tensor("w1f_s", [128, NB * 4 * 2 * 128], wdt)
        w1b = nc.alloc_sbuf_tensor("w1b_s", [128, NB * 2 * 4 * 128], wdt)
        w2f = nc.alloc_sbuf_tensor("w2f_s", [128, NB * 2 * 4 * 128], wdt)
        w2b = nc.alloc_sbuf_tensor("w2b_s", [128, NB * 4 * 2 * 128], wdt)
        xt = nc.alloc_sbuf_tensor("xt_s", [128, 2 * F], f32)
        s1m = nc.alloc_sbuf_tensor("s1m", [128, 4 * F], f32)
        s2m = nc.alloc_sbuf_tensor("s2m", [128, 2 * F], f32)
        s1pm = nc.alloc_sbuf_tensor("s1pm", [128, 4 * 128], wdt)
        s2pm = nc.alloc_sbuf_tensor("s2pm", [128, 2 * 128], wdt)
        e1pm = nc.alloc_sbuf_tensor("e1pm", [128, 2 * 128], wdt)
        e2pm = nc.alloc_sbuf_tensor("e2pm", [128, 4 * 128], wdt)

        def wslice(t, c, nj, mi, njc, mic):
            off = ((c * njc + nj) * mic + mi) * 128
            return t[:, off : off + 128]

        # x first, as a single DMA (one queue-sem tick for the DVE probe below)
        nc.sync.dma_start(
            out=xt[:].rearrange("p (n f) -> p n f", f=F),
            in_=xt_d.ap().transpose([1, 0, 2]),
        )
        # DVE probe: absorbs the DMA wait into the vector engine's clock so
        # later DVE ops touching xt need only one (PE) wait -- the ISA
        # TensorTensor slot fits a single sync-wait command.
        probe = nc.alloc_sbuf_tensor("probe", [128, 1], mybir.dt.float32)
        p_op = nc.vector.tensor_copy(out=probe[:], in_=xt[:, 0:1])

        # weight DMAs: one 256KB transfer per (tensor, blade), issue spread
        # across 4 engine queues so the issue rate doesn't serialize the
        # first iterations.
        dma_engines = [nc.sync, nc.gpsimd, nc.scalar]
        di = 0
        # first-use order: iteration 0 needs only the backward weights
        for d, s, njc, mic in (
            (w1b_d, w1b, 2, 4),
            (w2b_d, w2b, 4, 2),
            (w1f_d, w1f, 4, 2),
            (w2f_d, w2f, 2, 4),
        ):
            blkc = njc * mic * 128
            for c in range(NB):
                dma_engines[di % 3].dma_start(
                    out=s[:, c * blkc : (c + 1) * blkc].rearrange(
                        "p (n m q) -> p n m q", n=njc, m=mic
                    ),
                    in_=d.ap()[c].transpose([2, 0, 1, 3]),
                )
                di += 1

        for t in (s1m, s2m, s1pm, s2pm):
            ms = nc.vector.memset(t[:], 0.0)
            add_dep_helper(ms.ins, p_op.ins, sync=False, reason="probe first")

        def blk(t, mi):  # 64-col fp32 block
            return t[:, mi * F : (mi + 1) * F]

        def pmp(t, mi):  # + half of a +/- block
            return t[:, mi * 128 : mi * 128 + 64]

        def pmm(t, mi):  # - half
            return t[:, mi * 128 + 64 : mi * 128 + 128]

        xh2n = nc.alloc_sbuf_tensor("xh2n", [128, 4 * F], f32)  # -xhat2
        MUL, ADD, SUB = (
            mybir.AluOpType.mult,
            mybir.AluOpType.add,
            mybir.AluOpType.subtract,
        )

        with tc.tile_pool(name="ps", bufs=2, space="PSUM") as pp:
            for it in range(n_iter):
                first_it = it == 0
                # ---- xhat1 = gp(S1, w1f); err1 = x - xhat1  (S1=0 on iter 0)
                if first_it:
                    for mi in range(2):
                        nc.vector.tensor_copy(out=pmp(e1pm, mi), in_=blk(xt, mi))
                        nc.vector.tensor_scalar_mul(pmm(e1pm, mi), blk(xt, mi), -1.0)
                else:
                    for mi in range(2):
                        ps = pp.tile([128, F], f32, tag="xh1")
                        emit_gp(
                            nc,
                            ps[:],
                            s1pm[:],
                            lambda c, nj: wslice(w1f, c, nj, mi, 4, 2),
                            4,
                        )
                        nc.vector.tensor_sub(pmp(e1pm, mi), blk(xt, mi), ps[:])
                        nc.vector.tensor_sub(pmm(e1pm, mi), ps[:], blk(xt, mi))
                    # ---- xhat2 = gp(S2, w2f), evacuated as -xhat2 (PE stays
                    # busy here while DVE computes err1)
                    for mi in range(4):
                        ps = pp.tile([128, F], f32, tag="xh2")
                        emit_gp(
                            nc,
                            ps[:],
                            s2pm[:],
                            lambda c, nj: wslice(w2f, c, nj, mi, 2, 4),
                            2,
                        )
                        nc.vector.tensor_scalar_mul(blk(xh2n, mi), ps[:], -1.0)
                # ---- S1 += alpha*gp(err1, w1bT); s1pm = +/- new S1 (fused,
                # all three ops depend only on (psum, old S1))
                for mi in range(4):
                    ps = pp.tile([128, F], f32, tag="ds1")
                    emit_gp(
                        nc, ps[:], e1pm[:], lambda c, nj: wslice(w1b, c, nj, mi, 2, 4), 2
                    )
                    nc.vector.scalar_tensor_tensor(
                        out=pmp(s1pm, mi), in0=ps[:], scalar=1.0, in1=blk(s1m, mi),
                        op0=MUL, op1=ADD,
                    )
                    nc.vector.scalar_tensor_tensor(
                        out=pmm(s1pm, mi), in0=ps[:], scalar=-1.0, in1=blk(s1m, mi),
                        op0=MUL, op1=SUB,
                    )
                    nc.vector.tensor_add(blk(s1m, mi), blk(s1m, mi), ps[:])
                # ---- err2 = S1 - xhat2 (on iter 0 xhat2=0 so e2 == s1pm)
                if first_it:
                    e2src = s1pm
                else:
                    e2src = e2pm
                    for mi in range(4):
                        nc.vector.tensor_add(
                            pmp(e2pm, mi), blk(s1m, mi), blk(xh2n, mi)
                        )
                        nc.vector.scalar_tensor_tensor(
                            out=pmm(e2pm, mi), in0=blk(s1m, mi), scalar=-1.0,
                            in1=blk(xh2n, mi), op0=MUL, op1=SUB,
                        )
                # ---- S2 += alpha*gp(err2, w2bT); s2pm = +/- new S2
                for mi in range(2):
                    ps = pp.tile([128, F], f32, tag="ds2")
                    emit_gp(
                        nc,
                        ps[:],
                        e2src[:],
                        lambda c, nj: wslice(w2b, c, nj, mi, 4, 2),
                        4,
                    )
                    nc.vector.scalar_tensor_tensor(
                        out=pmp(s2pm, mi), in0=ps[:], scalar=1.0, in1=blk(s2m, mi),
                        op0=MUL, op1=ADD,
                    )
                    nc.vector.scalar_tensor_tensor(
                        out=pmm(s2pm, mi), in0=ps[:], scalar=-1.0, in1=blk(s2m, mi),
                        op0=MUL, op1=SUB,
                    )
                    nc.vector.tensor_add(blk(s2m, mi), blk(s2m, mi), ps[:])

        for nj in range(4):
            nc.sync.dma_start(out=s1o_d[nj], in_=blk(s1m, nj))
        for nj in range(2):
            nc.sync.dma_start(out=s2o_d[nj], in_=blk(s2m, nj))

    nc.compile()
    return nc


# ------------------------------------------------------------ host wrappers

_CACHE = {}


def _get_program(n_iter, wdt):
    key = (n_iter, wdt)
    if key not in _CACHE:
        _CACHE[key] = build_program(n_iter, wdt)
    return _CACHE[key]


def _prep_weights(w1, w2, npdt):
    a = np.float32(ALPHA)
    # W1f[c,nj,mi,p,q] = w1[mi*128+q, nj*128+p, c]
    w1f = np.ascontiguousarray(
        w1.transpose(2, 1, 0).reshape(NB, 4, 128, 2, 128).transpose(0, 1, 3, 2, 4)
    )
    # W1b[c,nj,mi,p,q] = a*REV[c]*w1[nj*128+p, mi*128+q, c]
    w1b = (a * REV)[:, None, None, None, None] * w1.transpose(2, 0, 1).reshape(
        NB, 2, 128, 4, 128
    ).transpose(0, 1, 3, 2, 4)
    # W2f[c,nj,mi,p,q] = w2[mi*128+q, nj*128+p, c]
    w2f = np.ascontiguousarray(
        w2.transpose(2, 1, 0).reshape(NB, 2, 128, 4, 128).transpose(0, 1, 3, 2, 4)
    )
    # W2b[c,nj,mi,p,q] = a*REV[c]*w2[nj*128+p, mi*128+q, c]
    w2b = (a * REV)[:, None, None, None, None] * w2.transpose(2, 0, 1).reshape(
        NB, 4, 128, 2, 128
    ).transpose(0, 1, 3, 2, 4)
    return [np.ascontiguousarray(t).astype(npdt) for t in (w1f, w1b, w2f, w2b)]


def _run(x, w1, w2, n_iter, trace=False, use_bf16=True):
    x = np.asarray(x, np.float32)
    w1 = np.asarray(w1, np.float32)
    w2 = np.asarray(w2, np.float32)
    n_iter = int(np.asarray(n_iter))

    wdt = mybir.dt.bfloat16 if use_bf16 else mybir.dt.float32
    npdt = ml_dtypes.bfloat16 if use_bf16 else np.float32
    nc = _get_program(n_iter, wdt)

    w1f, w1b, w2f, w2b = _prep_weights(w1, w2, npdt)

    in_maps = []
    for core in range(NCORES):
        xc = x[core * BLOC : (core + 1) * BLOC]          # (8, 256, 8)
        xtc = np.ascontiguousarray(
            xc.transpose(1, 2, 0).reshape(2, 128, F)
        )  # [nj,p,8k+b]
        in_maps.append(
            {"xt": xtc, "w1f": w1f, "w1b": w1b, "w2f": w2f, "w2b": w2b}
        )

    res = bass_utils.run_bass_kernel_spmd(
        nc, in_maps, core_ids=list(range(NCORES)), trace=trace
    )

    s1 = np.zeros((BATCH, D1, NB), np.float32)
    s2 = np.zeros((BATCH, D2, NB), np.float32)
    for core in range(NCORES):
        r = res.results[core]
        # s1o (4,128,64): [nj,p,8k+b] -> (b, nj*128+p, k)
        s1[core * BLOC : (core + 1) * BLOC] = (
            r["s1o"].reshape(4, 128, NB, BLOC).transpose(3, 0, 1, 2).reshape(BLOC, D1, NB)
        )
        s2[core * BLOC : (core + 1) * BLOC] = (
            r["s2o"].reshape(2, 128, NB, BLOC).transpose(3, 0, 1, 2).reshape(BLOC, D2, NB)
        )
    return (x, s1, s2), res


def kernel(x, w1, w2, n_iter):
    (x, s1, s2), _ = _run(x, w1, w2, n_iter)
    return (x, s1, s2)


# revision 20
# speedup vs baseline: 1.5933x; 1.1309x over previous
"""Clifford predictive-coding network on 8 TRN2 NeuronCores.

Problem: Cl(3,0) geometric-product PC net, layers [256, 512, 256], batch 64,
n_iter PC iterations:
    for l in 1,2:
        x_hat = gp(states[l], w_l)                 # contract over D_l
        err   = states[l-1] - x_hat
        states[l] += alpha * gp(err, transpose(w_l * REV))

Sharding: pure data-parallel over batch (8 cores x batch-8); weights and the
Cayley table are replicated; no cross-core communication.

Per-core kernel design ("arrangement A"):
  Every tensor lives in layout L1 = (feature dim on partitions in 128-chunks,
  64 free columns, col = 8*k + b) for blade k in 0..7, local batch b in 0..7.
  A geometric product out[b,m,k] = sum_{n,a,c} C[a,c,k] A[b,n,a] W[m,n,c]
  becomes, for each blade c and each 128-chunk (nj, mi):
      psum[mi][:, view] += lhsT(W~[c,nj,mi]).T @ rhs_view(state, c, nj)
  where rhs_view is an access-pattern view of the state tile that applies the
  XOR column permutation k -> k^c and the Clifford sign: the state is stored
  as [+S | -S] (128 cols per chunk) and the sign s(k,c) becomes a +64 column
  offset.  sign(k^c,c) = sigma(c) * (-1)^{k . u(c)} is affine in the bits of
  k for 6 of 8 blades; blades needing >3 AP dims (ISA limit) are split into
  two half-matmuls by k2 (contiguous psum halves).

  Master states are kept in fp32; matmul operands are bf16; alpha and the
  reversion signs are folded into the backward weight tiles on the host.
"""

import numpy as np
import ml_dtypes

import concourse.bass as bass
import concourse.mybir as mybir
import concourse.tile as tile
from concourse import bacc
from concourse.tile import add_dep_helper
from concourse import bass_utils

# ---------------------------------------------------------------- problem dims
NB = 8          # blades of Cl(3,0)
D0, D1, D2 = 256, 512, 256
BATCH = 64
NCORES = 8
BLOC = BATCH // NCORES          # 8 local batch
ALPHA = 0.1
F = 64                          # free columns: 8 blades x 8 batch


def _cayley():
    C = np.zeros((NB, NB, NB), dtype=np.float32)
    for a in range(NB):
        for b in range(NB):
            s, aa = 0, a >> 1
            while aa:
                s += bin(aa & b).count("1")
                aa >>= 1
            C[a, b, a ^ b] = -1.0 if (s & 1) else 1.0
    return C


CAYLEY = _cayley()
REV = np.array(
    [(-1.0) ** (bin(k).count("1") * (bin(k).count("1") - 1) // 2) for k in range(NB)],
    np.float32,
)

# ------------------------------------------------------- AP view construction


def _fit_affine(addr, klist):
    a = [addr[k] for k in klist]
    if len(klist) == 8:
        s0, s1, s2 = a[1] - a[0], a[2] - a[0], a[4] - a[0]
        for i in range(8):
            if a[i] != a[0] + ((i >> 2) & 1) * s2 + ((i >> 1) & 1) * s1 + (i & 1) * s0:
                return None
        return [(s2, 2), (s1, 2), (s0, 2)]
    s0, sg = a[1] - a[0], a[2] - a[0]
    for i in range(4):
        if a[i] != a[0] + ((i >> 1) & 1) * sg + (i & 1) * s0:
            return None
    return [(sg, 2), (s0, 2)]


def _merge(dims):
    out = list(dims)
    changed = True
    while changed:
        changed = False
        for i in range(len(out) - 1):
            (so, no), (si, ni) = out[i], out[i + 1]
            if so == si * ni:
                out[i : i + 2] = [(si, no * ni)]
                changed = True
                break
    return out


def gp_view_specs(c):
    """views for blade c: list of (src_off, src_dims, out_off, out_len).

    State blocks are 256 cols: [f0+ | f0- | f1+ | f1-], where the f1 copy
    stores columns in k^1 order.  Blade c reads variant f = c&1, so the
    effective XOR permutation never flips k0 and (k0, b) merges into one
    (1,16) dim; only blades {1,5} (non-affine sign parity k1^k2) split.
    """
    f = c & 1
    s = [1 if CAYLEY[k ^ c, c, k] < 0 else 0 for k in range(NB)]
    addr = {k: 128 * f + 64 * s[k] + 8 * ((k ^ c) ^ f) for k in range(NB)}
    full = _fit_affine(addr, list(range(8)))
    if full is not None:
        dims = _merge(list(full) + [(1, 8)])
        if len(dims) <= 3:
            return [(addr[0], dims, 0, 64)]
    views = []
    for h in (0, 1):
        klist = [4 * h + j for j in range(4)]
        sf = _fit_affine(addr, klist)
        assert sf is not None
        dims = _merge(list(sf) + [(1, 8)])
        assert len(dims) <= 3, (c, h, dims)
        views.append((addr[klist[0]], dims, 32 * h, 32))
    return views


VIEW_SPECS = [gp_view_specs(c) for c in range(NB)]
BLK_W = 256  # state block width: [f0+ | f0- | f1+ | f1-]
# f1 = adjacent-8-col-block swap of f0: one 3-dim AP copy
SWAP_DIMS = [(16, 4), (-8, 2), (1, 8)]
SWAP_OFF = 8


def _make_ap(base_ap, extra_off, dims):
    ap_pairs = [list(base_ap.ap[0])] + [[st, n] for (st, n) in dims]
    return bass.AP(
        tensor=base_ap.tensor, offset=base_ap.offset + extra_off, ap=ap_pairs
    )


def emit_gp(nc, ps_ap, st_ap, lhsT_of, njc):
    """psum (128,64) <- sum over (nj, c) of signed-permuted matmuls.

    st_ap: base AP of the +/- state tile (128, njc*128); block nj at col nj*128.
    lhsT_of(c, nj): (128,128) weight slice AP.
    """
    specs = [
        (c, nj, v)
        for par in (0, 1)
        for nj in range(njc)
        for c in range(par, NB, 2)
        for v in VIEW_SPECS[c]
    ]
    n = len(specs)
    for i, (c, nj, (soff, sdims, ooff, olen)) in enumerate(specs):
        rhs = _make_ap(st_ap, nj * BLK_W + soff, sdims)
        nc.tensor.matmul(
            ps_ap[:, ooff : ooff + olen],
            lhsT_of(c, nj),
            rhs,
            start=(i == 0),
            stop=(i == n - 1),
        )


# ------------------------------------------------------------- bass program


def _dedup_split_ldweights(nc):
    """Drop back-to-back InstLdweights with identical weight APs.

    The two half-matmuls of a split blade load the same stationary tile; the
    second load is redundant (the PE keeps the loaded weights across matmuls
    and walrus's ldw-opt is disabled).  Only waitless/updateless duplicates
    are dropped so semaphore bookkeeping is untouched.
    """
    removed = 0
    for blk in nc.m.functions[0].blocks:
        insts = blk.instructions
        new = []
        last_key = None
        for ins in insts:
            nm = type(ins).__name__
            if getattr(ins, "engine", None) == mybir.EngineType.PE:
                if nm == "InstLdweights":
                    ap = ins.ins[0]
                    key = (getattr(ap, "offset", None), str(getattr(ap, "ap", "")))
                    si = ins.sync_info
                    nw = len(si.on_wait) if si else 0
                    nu = len(si.on_update) if si else 0
                    if key == last_key and key[0] is not None and nw == 0 and nu == 0:
                        removed += 1
                        continue
                    last_key = key
                elif nm != "InstMatmult":
                    last_key = None
            new.append(ins)
        if len(new) != len(insts):
            insts[:] = new
    return removed


def build_program(n_iter, wdt=mybir.dt.bfloat16):
    f32 = mybir.dt.float32
    nc = bacc.Bacc("TRN2", target_bir_lowering=False, debug=False)

    # dram I/O (per-core shapes)
    xt_d = nc.dram_tensor("xt", (2, 128, F), f32, kind="ExternalInput")
    w1f_d = nc.dram_tensor("w1f", (NB, 4, 2, 128, 128), wdt, kind="ExternalInput")
    w1b_d = nc.dram_tensor("w1b", (NB, 2, 4, 128, 128), wdt, kind="ExternalInput")
    w2f_d = nc.dram_tensor("w2f", (NB, 2, 4, 128, 128), wdt, kind="ExternalInput")
    w2b_d = nc.dram_tensor("w2b", (NB, 4, 2, 128, 128), wdt, kind="ExternalInput")
    s1o_d = nc.dram_tensor("s1o", (4, 128, F), f32, kind="ExternalOutput")
    s2o_d = nc.dram_tensor("s2o", (2, 128, F), f32, kind="ExternalOutput")

    with tile.TileContext(nc) as tc:
        # resident sbuf tensors
        w1f = nc.alloc_sbuf_tensor("w1f_s", [128, NB * 4 * 2 * 128], wdt)
        w1b = nc.alloc_sbuf_tensor("w1b_s", [128, NB * 2 * 4 * 128], wdt)
        w2f = nc.alloc_sbuf_tensor("w2f_s", [128, NB * 2 * 4 * 128], wdt)
        w2b = nc.alloc_sbuf_tensor("w2b_s", [128, NB * 4 * 2 * 128], wdt)
        xt = nc.alloc_sbuf_tensor("xt_s", [128, 2 * F], f32)
        s1m = nc.alloc_sbuf_tensor("s1m", [128, 4 * F], f32)
        s2m = nc.alloc_sbuf_tensor("s2m", [128, 2 * F], f32)
        s1pm = nc.alloc_sbuf_tensor("s1pm", [128, 4 * BLK_W], wdt)
        s2pm = nc.alloc_sbuf_tensor("s2pm", [128, 2 * BLK_W], wdt)
        e1pm = nc.alloc_sbuf_tensor("e1pm", [128, 2 * BLK_W], wdt)
        e2pm = nc.alloc_sbuf_tensor("e2pm", [128, 4 * BLK_W], wdt)

        def wslice(t, c, nj, mi, njc, mic):
            off = ((c * njc + nj) * mic + mi) * 128
            return t[:, off : off + 128]

        # x first, as a single DMA (one queue-sem tick for the DVE probe below)
        nc.sync.dma_start(
            out=xt[:].rearrange("p (n f) -> p n f", f=F),
            in_=xt_d.ap().transpose([1, 0, 2]),
        )
        # DVE probe: absorbs the DMA wait into the vector engine's clock so
        # later DVE ops touching xt need only one (PE) wait -- the ISA
        # TensorTensor slot fits a single sync-wait command.
        probe = nc.alloc_sbuf_tensor("probe", [128, 1], mybir.dt.float32)
        p_op = nc.vector.tensor_copy(out=probe[:], in_=xt[:, 0:1])

        # weight DMAs: one 256KB transfer per (tensor, blade), issue spread
        # across 4 engine queues so the issue rate doesn't serialize the
        # first iterations.
        dma_engines = [nc.sync, nc.gpsimd, nc.scalar]
        di = 0
        # first-use order: iteration 0 needs only the backward weights
        for d, s, njc, mic in (
            (w1b_d, w1b, 2, 4),
            (w2b_d, w2b, 4, 2),
            (w1f_d, w1f, 4, 2),
            (w2f_d, w2f, 2, 4),
        ):
            blkc = njc * mic * 128
            for c in range(NB):
                dma_engines[di % 3].dma_start(
                    out=s[:, c * blkc : (c + 1) * blkc].rearrange(
                        "p (n m q) -> p n m q", n=njc, m=mic
                    ),
                    in_=d.ap()[c].transpose([2, 0, 1, 3]),
                )
                di += 1

        for t in (s1m, s2m, s1pm, s2pm):
            ms = nc.vector.memset(t[:], 0.0)
            add_dep_helper(ms.ins, p_op.ins, sync=False, reason="probe first")

        def blk(t, mi):  # 64-col fp32 block
            return t[:, mi * F : (mi + 1) * F]

        def pmp(t, mi):  # f0+ quarter of a block
            return t[:, mi * BLK_W : mi * BLK_W + 64]

        def pmm(t, mi):  # f0- quarter
            return t[:, mi * BLK_W + 64 : mi * BLK_W + 128]

        fct = [0]

        def fvar(t, mi, early=False):
            # write the f1+/f1- quarters as block-swapped copies of f0+/f0-,
            # alternating between otherwise-idle engines (gpsimd still issues
            # weight DMAs during the first iterations, so avoid it early)
            base = mi * BLK_W
            for half in (0, 64):
                src_ap = _make_ap(t[:, 0:1], base + half + SWAP_OFF, SWAP_DIMS)
                out_ap = _make_ap(
                    t[:, 0:1], base + 128 + half, [(16, 4), (8, 2), (1, 8)]
                )
                if fct[0] % 2 == 0:
                    nc.scalar.copy(out=out_ap, in_=src_ap)
                elif early:
                    nc.vector.tensor_copy(out=out_ap, in_=src_ap)
                else:
                    nc.gpsimd.tensor_copy(out=out_ap, in_=src_ap)
                fct[0] += 1

        xh2n = nc.alloc_sbuf_tensor("xh2n", [128, 4 * F], f32)  # -xhat2
        MUL, ADD, SUB = (
            mybir.AluOpType.mult,
            mybir.AluOpType.add,
            mybir.AluOpType.subtract,
        )

        with tc.tile_pool(name="ps", bufs=2, space="PSUM") as pp:
            for it in range(n_iter):
                first_it = it == 0
                # ---- xhat1 = gp(S1, w1f); err1 = x - xhat1  (S1=0 on iter 0)
                if first_it:
                    for mi in range(2):
                        nc.vector.tensor_copy(out=pmp(e1pm, mi), in_=blk(xt, mi))
                        nc.vector.tensor_scalar_mul(pmm(e1pm, mi), blk(xt, mi), -1.0)
                        fvar(e1pm, mi, early=True)
                else:
                    for mi in range(2):
                        ps = pp.tile([128, F], f32, tag="xh1")
                        emit_gp(
                            nc,
                            ps[:],
                            s1pm[:],
                            lambda c, nj: wslice(w1f, c, nj, mi, 4, 2),
                            4,
                        )
                        nc.vector.tensor_sub(pmp(e1pm, mi), blk(xt, mi), ps[:])
                        nc.vector.tensor_sub(pmm(e1pm, mi), ps[:], blk(xt, mi))
                        fvar(e1pm, mi, early=(it < 6))
                    # ---- xhat2 = gp(S2, w2f), evacuated as -xhat2 (PE stays
                    # busy here while DVE computes err1)
                    for mi in range(4):
                        ps = pp.tile([128, F], f32, tag="xh2")
                        emit_gp(
                            nc,
                            ps[:],
                            s2pm[:],
                            lambda c, nj: wslice(w2f, c, nj, mi, 2, 4),
                            2,
                        )
                        nc.scalar.mul(blk(xh2n, mi), ps[:], -1.0)
                # ---- S1 += alpha*gp(err1, w1bT); s1pm = +/- new S1 (fused,
                # all three ops depend only on (psum, old S1))
                for mi in range(4):
                    ps = pp.tile([128, F], f32, tag="ds1")
                    emit_gp(
                        nc, ps[:], e1pm[:], lambda c, nj: wslice(w1b, c, nj, mi, 2, 4), 2
                    )
                    nc.vector.scalar_tensor_tensor(
                        out=pmp(s1pm, mi), in0=ps[:], scalar=1.0, in1=blk(s1m, mi),
                        op0=MUL, op1=ADD,
                    )
                    nc.vector.scalar_tensor_tensor(
                        out=pmm(s1pm, mi), in0=ps[:], scalar=-1.0, in1=blk(s1m, mi),
                        op0=MUL, op1=SUB,
                    )
                    fvar(s1pm, mi, early=(it < 6))
                    nc.vector.tensor_add(blk(s1m, mi), blk(s1m, mi), ps[:])
                # ---- err2 = S1 - xhat2 (on iter 0 xhat2=0 so e2 == s1pm)
                if first_it:
                    e2src = s1pm
                else:
                    e2src = e2pm
                    for mi in range(4):
                        nc.vector.tensor_add(
                            pmp(e2pm, mi), blk(s1m, mi), blk(xh2n, mi)
                        )
                        nc.vector.scalar_tensor_tensor(
                            out=pmm(e2pm, mi), in0=blk(s1m, mi), scalar=-1.0,
                            in1=blk(xh2n, mi), op0=MUL, op1=SUB,
                        )
                        fvar(e2pm, mi, early=(it < 6))
                # ---- S2 += alpha*gp(err2, w2bT); s2pm = +/- new S2
                for mi in range(2):
                    ps = pp.tile([128, F], f32, tag="ds2")
                    emit_gp(
                        nc,
                        ps[:],
                        e2src[:],
                        lambda c, nj: wslice(w2b, c, nj, mi, 4, 2),
                        4,
                    )
                    nc.vector.scalar_tensor_tensor(
                        out=pmp(s2pm, mi), in0=ps[:], scalar=1.0, in1=blk(s2m, mi),
                        op0=MUL, op1=ADD,
                    )
                    nc.vector.scalar_tensor_tensor(
                        out=pmm(s2pm, mi), in0=ps[:], scalar=-1.0, in1=blk(s2m, mi),
                        op0=MUL, op1=SUB,
                    )
                    fvar(s2pm, mi, early=(it < 6))
                    nc.vector.tensor_add(blk(s2m, mi), blk(s2m, mi), ps[:])

        for nj in range(4):
            nc.sync.dma_start(out=s1o_d[nj], in_=blk(s1m, nj))
        for nj in range(2):
            nc.sync.dma_start(out=s2o_d[nj], in_=blk(s2m, nj))

    _dedup_split_ldweights(nc)
    nc.compile()
    return nc


# ------------------------------------------------------------ host wrappers

_CACHE = {}


def _get_program(n_iter, wdt):
    key = (n_iter, wdt)
    if key not in _CACHE:
        _CACHE[key] = build_program(n_iter, wdt)
    return _CACHE[key]


def _prep_weights(w1, w2, npdt):
    a = np.float32(ALPHA)
    # W1f[c,nj,mi,p,q] = w1[mi*128+q, nj*128+p, c]
    w1f = np.ascontiguousarray(
        w1.transpose(2, 1, 0).reshape(NB, 4, 128, 2, 128).transpose(0, 1, 3, 2, 4)
    )
    # W1b[c,nj,mi,p,q] = a*REV[c]*w1[nj*128+p, mi*128+q, c]
    w1b = (a * REV)[:, None, None, None, None] * w1.transpose(2, 0, 1).reshape(
        NB, 2, 128, 4, 128
    ).transpose(0, 1, 3, 2, 4)
    # W2f[c,nj,mi,p,q] = w2[mi*128+q, nj*128+p, c]
    w2f = np.ascontiguousarray(
        w2.transpose(2, 1, 0).reshape(NB, 2, 128, 4, 128).transpose(0, 1, 3, 2, 4)
    )
    # W2b[c,nj,mi,p,q] = a*REV[c]*w2[nj*128+p, mi*128+q, c]
    w2b = (a * REV)[:, None, None, None, None] * w2.transpose(2, 0, 1).reshape(
        NB, 4, 128, 2, 128
    ).transpose(0, 1, 3, 2, 4)
    return [np.ascontiguousarray(t).astype(npdt) for t in (w1f, w1b, w2f, w2b)]


def _run(x, w1, w2, n_iter, trace=False, use_bf16=True):
    x = np.asarray(x, np.float32)
    w1 = np.asarray(w1, np.float32)
    w2 = np.asarray(w2, np.float32)
    n_iter = int(np.asarray(n_iter))

    wdt = mybir.dt.bfloat16 if use_bf16 else mybir.dt.float32
    npdt = ml_dtypes.bfloat16 if use_bf16 else np.float32
    nc = _get_program(n_iter, wdt)

    w1f, w1b, w2f, w2b = _prep_weights(w1, w2, npdt)

    in_maps = []
    for core in range(NCORES):
        xc = x[core * BLOC : (core + 1) * BLOC]          # (8, 256, 8)
        xtc = np.ascontiguousarray(
            xc.transpose(1, 2, 0).reshape(2, 128, F)
        )  # [nj,p,8k+b]
        in_maps.append(
            {"xt": xtc, "w1f": w1f, "w1b": w1b, "w2f": w2f, "w2b": w2b}
        )

    res = bass_utils.run_bass_kernel_spmd(
        nc, in_maps, core_ids=list(range(NCORES)), trace=trace
    )

    s1 = np.zeros((BATCH, D1, NB), np.float32)
    s2 = np.zeros((BATCH, D2, NB), np.float32)
    for core in range(NCORES):
        r = res.results[core]
        # s1o (4,128,64): [nj,p,8k+b] -> (b, nj*128+p, k)
        s1[core * BLOC : (core + 1) * BLOC] = (
            r["s1o"].reshape(4, 128, NB, BLOC).transpose(3, 0, 1, 2).reshape(BLOC, D1, NB)
        )
        s2[core * BLOC : (core + 1) * BLOC] = (
            r["s2o"].reshape(2, 128, NB, BLOC).transpose(3, 0, 1, 2).reshape(BLOC, D2, NB)
        )
    return (x, s1, s2), res


def kernel(x, w1, w2, n_iter):
    (x, s1, s2), _ = _run(x, w1, w2, n_iter)
    return (x, s1, s2)
